# revision 10
# baseline (speedup 1.0000x reference)
"""Trainium2 Bass kernel for nn_MetricLoss (pairwise-distance metric loss).

Computation (reference):
    f = x.reshape(1024, 49152)
    G = f @ f.T                      (103 GFLOP Gram matrix)
    dist = sq_i + sq_j - 2 G         (relu never binds off-diagonal)
    loss_homo  = 0.5 * sum(same-group dist)
    loss_heter = sum(cross-group relu(1 - dist))

Distribution (8 NeuronCores, one TRN2 chip):
    K-parallel: core c holds f[:, c*6144:(c+1)*6144].T as a [48, 128, 1024]
    bf16 tensor (k-major tiles, fully SBUF-resident). Each core computes a
    partial Gram [1024, 1024] over its K-slice via PSUM-accumulated matmuls:
    8 row-block chains x 4 uneven column chunks (384/256/256/128 wide; the
    wide first chunk hides the input DMA, the narrow last chunk minimizes
    the exposed tail collective). Partial Grams are reduced with 4 chunked
    bf16 ReduceScatters so core c ends up with full-K Gram rows
    [128c:128c+128]. Row norms sq (= Gram diagonal) are extracted on-device
    with masked reduces and packed per-chunk as extra bf16 columns of the
    same ReduceScatters. A fused DVE epilogue computes the masked hinge
    sums; the host sums 8x[128,2] partials and normalizes.
"""

import numpy as np
import ml_dtypes

import concourse.bass as bass
import concourse.bacc as bacc
import concourse.tile as tile
import concourse.mybir as mybir
from concourse import bass_utils

F32 = mybir.dt.float32
BF16 = mybir.dt.bfloat16
ALU = mybir.AluOpType

N_CORES = 8
N = 1024            # batch (rows of f)
K = 64 * 768        # 49152 features per sample
KC = K // N_CORES   # 6144 features per core
KT = KC // 128      # 48 k-tiles of 128 per core
BK = 8              # samples per class group
MB = N // 128       # 8 row blocks

CWS = [384, 256, 256, 128]          # column-chunk widths
OFF = [0, 384, 640, 896]            # chunk column offsets
NJ = len(CWS)
# diag block b (cols 128b..128b+128) lives in chunk j where it fits
DIAGS = {0: [0, 1, 2], 1: [3, 4], 2: [5, 6], 3: [7]}

_CACHE = {}


def _build_nc():
    nc = bacc.Bacc("TRN2", target_bir_lowering=False, debug=False,
                   num_devices=N_CORES)

    ft = nc.dram_tensor("ft", [KT, 128, N], BF16, kind="ExternalInput").ap()
    mask_same = nc.dram_tensor("mask_same", [128, N], F32,
                               kind="ExternalInput").ap()
    mask_diff = nc.dram_tensor("mask_diff", [128, N], F32,
                               kind="ExternalInput").ap()
    # dm_big[i, 256 + i] = 1; slice [256-off : 256-off+W] puts the 1 at
    # local column off + i
    dm_big = nc.dram_tensor("dm_big", [128, 640], F32,
                            kind="ExternalInput").ap()
    emask = nc.dram_tensor("emask", [128, 8], BF16, kind="ExternalInput").ap()
    out = nc.dram_tensor("out", [128, 2], F32, kind="ExternalOutput").ap()

    rg = [list(range(N_CORES))]

    with tile.TileContext(nc) as tc:
        with (
            tc.tile_pool(name="ftp", bufs=1) as ftp,
            tc.tile_pool(name="misc", bufs=1) as misc,
            tc.tile_pool(name="gcopy", bufs=4) as gcp,
            tc.tile_pool(name="junk", bufs=2) as jkp,
            tc.tile_pool(name="psum", bufs=8, space="PSUM") as psp,
            tc.tile_pool(name="dram", bufs=1, space="DRAM") as drp,
        ):
            # ---- load inputs to SBUF ----
            ft_sb = []
            for k in range(KT):
                t = ftp.tile([128, N], BF16, tag=f"ft{k}", name=f"ft{k}")
                nc.sync.dma_start(t[:], ft[k])
                ft_sb.append(t)

            ms_sb = misc.tile([128, N], F32, tag="ms", name="ms")
            md_sb = misc.tile([128, N], F32, tag="md", name="md")
            dm_sb = misc.tile([128, 640], F32, tag="dm", name="dm")
            em_sb = misc.tile([128, 8], BF16, tag="em", name="em")
            nc.sync.dma_start(ms_sb[:], mask_same[:])
            nc.sync.dma_start(md_sb[:], mask_diff[:])
            nc.sync.dma_start(dm_sb[:], dm_big[:])
            nc.sync.dma_start(em_sb[:], emask[:])

            # sq partials per row block: sqp[:, b] = diag of block b
            sqp = misc.tile([128, 8], F32, tag="sqp", name="sqp")
            sqp_bf = misc.tile([128, 8], BF16, tag="sqpb", name="sqpb")

            bounce = []
            rs = []
            for j in range(NJ):
                w = CWS[j] + len(DIAGS[j])
                bounce.append(drp.tile([N, w], BF16, tag=f"bnc{j}",
                                       name=f"bnc{j}"))
                rs.append(drp.tile([128, w], BF16, tag=f"rs{j}",
                                   name=f"rs{j}"))

            # ---- partial Gram: 4 column chunks x 8 row-block chains ----
            for j in range(NJ):
                W = CWS[j]
                ne = len(DIAGS[j])
                chains = [psp.tile([128, W], F32, tag="chain",
                                   name=f"ch{j}_{m}") for m in range(MB)]
                for k in range(KT):
                    for m in range(MB):
                        nc.tensor.matmul(
                            chains[m][:],
                            lhsT=ft_sb[k][:, m * 128:(m + 1) * 128],
                            rhs=ft_sb[k][:, OFF[j]:OFF[j] + W],
                            start=(k == 0),
                            stop=(k == KT - 1),
                        )
                # diag extraction first (on the RS critical path for the
                # last chunk), then bf16 copies + bounce writes
                for e, b in enumerate(DIAGS[j]):
                    off = 128 * b - OFF[j]
                    junk = jkp.tile([128, W], F32, tag="jk",
                                    name=f"jk{j}_{b}")
                    nc.vector.tensor_tensor(
                        junk[:], chains[b][:],
                        dm_sb[:, 256 - off:256 - off + W], ALU.mult)
                    nc.vector.reduce_sum(sqp[:, b:b + 1], junk[:],
                                         axis=mybir.AxisListType.X)
                bs = DIAGS[j]
                nc.vector.tensor_copy(sqp_bf[:, bs[0]:bs[-1] + 1],
                                      sqp[:, bs[0]:bs[-1] + 1])
                # pack this chunk's sq blocks as extra cols of its bounce
                for cp in range(MB):
                    nc.sync.dma_start(
                        bounce[j][cp * 128:(cp + 1) * 128, W:W + ne],
                        sqp_bf[:, bs[0]:bs[-1] + 1])
                for m in range(MB):
                    g = gcp.tile([128, W], BF16, tag="g", name=f"g{j}_{m}")
                    nc.vector.tensor_copy(g[:], chains[m][:])
                    nc.sync.dma_start(
                        bounce[j][m * 128:(m + 1) * 128, 0:W], g[:])
                nc.gpsimd.collective_compute(
                    "ReduceScatter",
                    ALU.add,
                    replica_groups=rg,
                    ins=[bounce[j].opt()],
                    outs=[rs[j].opt()],
                )

            # ---- epilogue ----
            # gram rows + sq assembly (fire per chunk as RS lands)
            G_sb = misc.tile([128, N], BF16, tag="G", name="G")
            S_all = misc.tile([128, 8], BF16, tag="S", name="S")
            flat = misc.tile([1, N], BF16, tag="flat", name="flat")
            for j in range(NJ):
                W = CWS[j]
                nc.sync.dma_start(G_sb[:, OFF[j]:OFF[j] + W], rs[j][:, 0:W])
                for e, b in enumerate(DIAGS[j]):
                    nc.sync.dma_start(S_all[:, b:b + 1],
                                      rs[j][:, W + e:W + e + 1])
                    nc.sync.dma_start(flat[0:1, b * 128:(b + 1) * 128],
                                      rs[j][:, W + e:W + e + 1])

            ones = misc.tile([1, 128], BF16, tag="ones", name="ones")
            nc.vector.memset(ones[:], 1.0)

            # t0_j = -2 * G_j and B_j = broadcast(sq_j) can start as soon as
            # chunk j's RS lands (overlaps the later chunks' RS)
            t0s = []
            Bs = []
            for j in range(NJ):
                W = CWS[j]
                t0 = jkp.tile([128, W], F32, tag="t0", name=f"t0_{j}")
                nc.vector.tensor_scalar_mul(t0[:], G_sb[:, OFF[j]:OFF[j] + W],
                                            -2.0)
                t0s.append(t0)
                Bj = psp.tile([128, W], F32, tag="chain", name=f"B{j}")
                nc.tensor.matmul(Bj[:], lhsT=ones[:],
                                 rhs=flat[0:1, OFF[j]:OFF[j] + W],
                                 start=True, stop=True)
                Bs.append(Bj)

            # sq_row[i] = sq[128*core + i] via emask selection
            sq_row = misc.tile([128, 1], F32, tag="sqr", name="sqr")
            junk8 = misc.tile([128, 8], F32, tag="jk8", name="junk8")
            nc.vector.tensor_tensor(junk8[:], S_all[:], em_sb[:], ALU.mult)
            nc.vector.reduce_sum(sq_row[:], junk8[:],
                                 axis=mybir.AxisListType.X)

            acc_h = []
            acc_e = []
            for j in range(NJ):
                W = CWS[j]
                sl = slice(OFF[j], OFF[j] + W)
                d = jkp.tile([128, W], F32, tag="d", name=f"d{j}")
                nc.vector.scalar_tensor_tensor(
                    out=d[:], in0=t0s[j][:], scalar=sq_row[:], in1=Bs[j][:],
                    op0=ALU.add, op1=ALU.add)
                ah = misc.tile([128, 1], F32, tag=f"ah{j}", name=f"ah{j}")
                jh = jkp.tile([128, W], F32, tag="jh", name=f"jh{j}")
                nc.vector.tensor_tensor(jh[:], d[:], ms_sb[:, sl], ALU.mult)
                nc.vector.reduce_sum(ah[:], jh[:], axis=mybir.AxisListType.X)
                acc_h.append(ah)
                # min(d-1, 0) = -relu(1-d); heter partial = -sum(mask * that)
                # (negation applied on the host)
                t1 = jkp.tile([128, W], F32, tag="t1", name=f"t1_{j}")
                nc.vector.tensor_scalar(
                    t1[:], d[:], -1.0, 0.0, ALU.add, ALU.min)
                eh = misc.tile([128, 1], F32, tag=f"eh{j}", name=f"eh{j}")
                je = jkp.tile([128, W], F32, tag="je", name=f"je{j}")
                nc.vector.tensor_tensor(je[:], t1[:], md_sb[:, sl], ALU.mult)
                nc.vector.reduce_sum(eh[:], je[:], axis=mybir.AxisListType.X)
                acc_e.append(eh)

            out_sb = misc.tile([128, 2], F32, tag="osb", name="osb")
            hs = misc.tile([128, 2], F32, tag="hs", name="hs")
            es = misc.tile([128, 2], F32, tag="es", name="es")
            nc.vector.tensor_tensor(hs[:, 0:1], acc_h[0][:], acc_h[1][:],
                                    ALU.add)
            nc.vector.tensor_tensor(hs[:, 1:2], acc_h[2][:], acc_h[3][:],
                                    ALU.add)
            nc.vector.tensor_tensor(es[:, 0:1], acc_e[0][:], acc_e[1][:],
                                    ALU.add)
            nc.vector.tensor_tensor(es[:, 1:2], acc_e[2][:], acc_e[3][:],
                                    ALU.add)
            nc.vector.tensor_tensor(out_sb[:, 0:1], hs[:, 0:1], hs[:, 1:2],
                                    ALU.add)
            nc.vector.tensor_tensor(out_sb[:, 1:2], es[:, 0:1], es[:, 1:2],
                                    ALU.add)
            nc.sync.dma_start(out[:], out_sb[:])

    nc.compile()
    return nc


def _host_inputs(x: np.ndarray):
    """Shard + transpose + cast x into per-core input maps."""
    f = np.ascontiguousarray(x.reshape(N, K))
    groups = np.arange(N) // BK
    cols = np.arange(N)

    dm = np.zeros((128, 640), dtype=np.float32)
    dm[np.arange(128), 256 + np.arange(128)] = 1.0

    in_maps = []
    for c in range(N_CORES):
        ftc = np.ascontiguousarray(
            f[:, c * KC:(c + 1) * KC].T).astype(ml_dtypes.bfloat16)
        rows = c * 128 + np.arange(128)
        g_r = groups[rows]
        same = ((g_r[:, None] == groups[None, :]) &
                (rows[:, None] != cols[None, :])).astype(np.float32)
        diff = (g_r[:, None] != groups[None, :]).astype(np.float32)
        em = np.zeros((128, 8), dtype=ml_dtypes.bfloat16)
        em[:, c] = 1.0
        in_maps.append({
            "ft": ftc.reshape(KT, 128, N),
            "mask_same": same,
            "mask_diff": diff,
            "dm_big": dm,
            "emask": em,
        })
    return in_maps


def kernel(x: np.ndarray):
    if "nc" not in _CACHE:
        _CACHE["nc"] = _build_nc()
    nc = _CACHE["nc"]

    in_maps = _host_inputs(x)
    res = bass_utils.run_bass_kernel_spmd(
        nc, in_maps, core_ids=list(range(N_CORES)))

    total_h = 0.0
    total_e = 0.0
    for c in range(N_CORES):
        o = res.results[c]["out"].astype(np.float64)
        total_h += o[:, 0].sum()
        total_e += o[:, 1].sum()

    # reference: 2 * (0.5 * sum_same dist) / (N * (BK - 1))
    #            2 * sum_diff relu(1 - dist) / (N * (N // BK - 1))
    # device accumulates sum(min(dist-1, 0) * mask_diff) = -heter partial
    homo = total_h / (N * (BK - 1))
    heter = -2.0 * total_e / (N * (N // BK - 1))
    return (np.float32(homo), np.float32(heter))


# revision 11
# speedup vs baseline: 1.2033x; 1.2033x over previous
"""Trainium2 Bass kernel for nn_MetricLoss (pairwise-distance metric loss).

Computation (reference):
    f = x.reshape(1024, 49152)
    G = f @ f.T                      (103 GFLOP Gram matrix)
    dist = sq_i + sq_j - 2 G         (the relu(dist) only binds on the
                                      diagonal, which both masks zero out)
    loss_homo  = 0.5 * sum(same-group dist)
    loss_heter = sum(cross-group relu(1 - dist))

Distribution (8 NeuronCores, one TRN2 chip):
    K-parallel: core c holds f[:, c*6144:(c+1)*6144].T as a [48, 128, 1024]
    bf16 tensor (k-major tiles, fully SBUF-resident). Each core computes a
    partial Gram [1024, 1024] over its K-slice via PSUM-accumulated matmuls:
    8 row-block chains x 4 uneven column chunks (384/256/256/128 wide; the
    wide first chunk hides the input DMA, the narrow last chunk minimizes
    the exposed tail collective). Partial Grams are reduced with 4 chunked
    bf16 ReduceScatters so core c ends up with full-K Gram rows
    [128c:128c+128]. Row norms sq are computed in fp32 on the otherwise-idle
    Scalar + Vector + GpSimd engines (square, accumulate over k, partition
    all-reduce) and summed across cores with a tiny fp32 AllReduce slotted
    into a gap in the collective queue. A fused DVE epilogue computes the
    masked hinge sums; the host sums 8x[128,2] partials and normalizes.
"""

import numpy as np
import ml_dtypes

import concourse.bass as bass
import concourse.bacc as bacc
import concourse.tile as tile
import concourse.mybir as mybir
import concourse.bass_isa as bass_isa
from concourse import bass_utils

F32 = mybir.dt.float32
BF16 = mybir.dt.bfloat16
ALU = mybir.AluOpType
AF = mybir.ActivationFunctionType

N_CORES = 8
N = 1024            # batch (rows of f)
K = 64 * 768        # 49152 features per sample
KC = K // N_CORES   # 6144 features per core
KT = KC // 128      # 48 k-tiles of 128 per core
BK = 8              # samples per class group
MB = N // 128       # 8 row blocks

CWS = [384, 256, 256, 128]          # column-chunk widths
OFF = [0, 384, 640, 896]            # chunk column offsets
NJ = len(CWS)

_CACHE = {}


def _build_nc():
    nc = bacc.Bacc("TRN2", target_bir_lowering=False, debug=False,
                   num_devices=N_CORES)

    ft = nc.dram_tensor("ft", [KT, 128, N], BF16, kind="ExternalInput").ap()
    mask_same = nc.dram_tensor("mask_same", [128, N], F32,
                               kind="ExternalInput").ap()
    mask_diff = nc.dram_tensor("mask_diff", [128, N], F32,
                               kind="ExternalInput").ap()
    emask = nc.dram_tensor("emask", [128, 8], F32, kind="ExternalInput").ap()
    out = nc.dram_tensor("out", [128, 2], F32, kind="ExternalOutput").ap()

    rg = [list(range(N_CORES))]

    with tile.TileContext(nc) as tc:
        with (
            tc.tile_pool(name="ftp", bufs=1) as ftp,
            tc.tile_pool(name="misc", bufs=1) as misc,
            tc.tile_pool(name="gcopy", bufs=4) as gcp,
            tc.tile_pool(name="sqt", bufs=3) as sqtp,
            tc.tile_pool(name="junk", bufs=2) as jkp,
            tc.tile_pool(name="psum", bufs=8, space="PSUM") as psp,
            tc.tile_pool(name="dram", bufs=1, space="DRAM") as drp,
        ):
            # ---- warmup collective (absorbs first-collective overhead) ----
            warm_sb = misc.tile([1, 8], F32, tag="wsb", name="warm_sb")
            nc.vector.memset(warm_sb[:], 0.0)
            warm_in = drp.tile([1, 8], F32, tag="wi", name="warm_in")
            warm_out = drp.tile([1, 8], F32, tag="wo", name="warm_out")
            nc.sync.dma_start(warm_in[:], warm_sb[:])
            nc.gpsimd.collective_compute(
                "AllReduce", ALU.add, replica_groups=rg,
                ins=[warm_in.opt()], outs=[warm_out.opt()])

            # ---- load inputs to SBUF ----
            ft_sb = []
            for k in range(KT):
                t = ftp.tile([128, N], BF16, tag=f"ft{k}", name=f"ft{k}")
                nc.sync.dma_start(t[:], ft[k])
                ft_sb.append(t)

            ms_sb = misc.tile([128, N], F32, tag="ms", name="ms")
            md_sb = misc.tile([128, N], F32, tag="md", name="md")
            em_sb = misc.tile([128, 8], F32, tag="em", name="em")
            nc.sync.dma_start(ms_sb[:], mask_same[:])
            nc.sync.dma_start(md_sb[:], mask_diff[:])
            nc.sync.dma_start(em_sb[:], emask[:])

            # ---- sq pipeline on ACT (square) + DVE (accumulate) ----
            acc = misc.tile([128, N], F32, tag="acc", name="acc")
            nc.vector.memset(acc[:], 0.0)
            for k in range(KT):
                sqt = sqtp.tile([128, N], F32, tag="sqt", name=f"sqt{k}")
                nc.scalar.activation(sqt[:], ft_sb[k][:], AF.Square)
                nc.vector.tensor_tensor(acc[:], acc[:], sqt[:], ALU.add)

            bounce = []
            rs = []
            for j in range(NJ):
                bounce.append(drp.tile([N, CWS[j]], BF16, tag=f"bnc{j}",
                                       name=f"bnc{j}"))
                rs.append(drp.tile([128, CWS[j]], BF16, tag=f"rs{j}",
                                   name=f"rs{j}"))
            sqb = drp.tile([1, N], F32, tag="sqb", name="sqb")
            sq_ar = drp.tile([1, N], F32, tag="sqar", name="sq_ar")

            # ---- partial Gram: 4 column chunks x 8 row-block chains ----
            for j in range(NJ):
                W = CWS[j]
                chains = [psp.tile([128, W], F32, tag="chain",
                                   name=f"ch{j}_{m}") for m in range(MB)]
                for k in range(KT):
                    for m in range(MB):
                        nc.tensor.matmul(
                            chains[m][:],
                            lhsT=ft_sb[k][:, m * 128:(m + 1) * 128],
                            rhs=ft_sb[k][:, OFF[j]:OFF[j] + W],
                            start=(k == 0),
                            stop=(k == KT - 1),
                        )
                for m in range(MB):
                    g = gcp.tile([128, W], BF16, tag="g", name=f"g{j}_{m}")
                    nc.vector.tensor_copy(g[:], chains[m][:])
                    nc.sync.dma_start(
                        bounce[j][m * 128:(m + 1) * 128, :], g[:])
                nc.gpsimd.collective_compute(
                    "ReduceScatter", ALU.add, replica_groups=rg,
                    ins=[bounce[j].opt()], outs=[rs[j].opt()])
                if j == 0:
                    # cross-partition reduce + fp32 AllReduce of sq, slotted
                    # into the collective-queue gap after RS0
                    par = misc.tile([128, N], F32, tag="par", name="par")
                    nc.gpsimd.partition_all_reduce(
                        par[:], acc[:], channels=128,
                        reduce_op=bass_isa.ReduceOp.add)
                    nc.sync.dma_start(sqb[:], par[0:1, :])
                    nc.gpsimd.collective_compute(
                        "AllReduce", ALU.add, replica_groups=rg,
                        ins=[sqb.opt()], outs=[sq_ar.opt()])
                    # sq_col broadcast [128, N] built right after the AR
                    flat_sb = misc.tile([1, N], F32, tag="flat", name="flat")
                    nc.sync.dma_start(flat_sb[:], sq_ar[:])
                    B_sb = misc.tile([128, N], F32, tag="B", name="B")
                    nc.gpsimd.partition_broadcast(B_sb[:], flat_sb[0:1, :],
                                                  channels=128)

            # sq_row[i] = sq[128*core + i] via emask selection
            S_all = misc.tile([128, 8], F32, tag="S", name="S")
            for b in range(MB):
                nc.sync.dma_start(S_all[:, b:b + 1],
                                  sq_ar[0:1, b * 128:(b + 1) * 128])
            sq_row = misc.tile([128, 1], F32, tag="sqr", name="sqr")
            junk8 = misc.tile([128, 8], F32, tag="jk8", name="junk8")
            nc.vector.tensor_tensor(junk8[:], S_all[:], em_sb[:], ALU.mult)
            nc.vector.reduce_sum(sq_row[:], junk8[:],
                                 axis=mybir.AxisListType.X)

            # ---- epilogue (chunks 0-2 run while RS3 is in flight) ----
            G_sb = misc.tile([128, N], BF16, tag="G", name="G")
            acc_h = []
            acc_e = []
            for j in range(NJ):
                W = CWS[j]
                sl = slice(OFF[j], OFF[j] + W)
                nc.sync.dma_start(G_sb[:, sl], rs[j][:, :])
                t0 = jkp.tile([128, W], F32, tag="t0", name=f"t0_{j}")
                nc.vector.tensor_scalar_mul(t0[:], G_sb[:, sl], -2.0)
                d = jkp.tile([128, W], F32, tag="d", name=f"d{j}")
                nc.vector.scalar_tensor_tensor(
                    out=d[:], in0=t0[:], scalar=sq_row[:], in1=B_sb[:, sl],
                    op0=ALU.add, op1=ALU.add)
                ah = misc.tile([128, 1], F32, tag=f"ah{j}", name=f"ah{j}")
                jh = jkp.tile([128, W], F32, tag="jh", name=f"jh{j}")
                nc.vector.tensor_tensor(jh[:], d[:], ms_sb[:, sl], ALU.mult)
                nc.vector.reduce_sum(ah[:], jh[:], axis=mybir.AxisListType.X)
                acc_h.append(ah)
                # min(d-1, 0) = -relu(1-d); heter partial = -sum(mask * that)
                # (negation applied on the host)
                t1 = jkp.tile([128, W], F32, tag="t1", name=f"t1_{j}")
                nc.vector.tensor_scalar(
                    t1[:], d[:], -1.0, 0.0, ALU.add, ALU.min)
                eh = misc.tile([128, 1], F32, tag=f"eh{j}", name=f"eh{j}")
                je = jkp.tile([128, W], F32, tag="je", name=f"je{j}")
                nc.vector.tensor_tensor(je[:], t1[:], md_sb[:, sl], ALU.mult)
                nc.vector.reduce_sum(eh[:], je[:], axis=mybir.AxisListType.X)
                acc_e.append(eh)

            out_sb = misc.tile([128, 2], F32, tag="osb", name="osb")
            hs = misc.tile([128, 2], F32, tag="hs", name="hs")
            es = misc.tile([128, 2], F32, tag="es", name="es")
            nc.vector.tensor_tensor(hs[:, 0:1], acc_h[0][:], acc_h[1][:],
                                    ALU.add)
            nc.vector.tensor_tensor(hs[:, 1:2], acc_h[2][:], acc_h[3][:],
                                    ALU.add)
            nc.vector.tensor_tensor(es[:, 0:1], acc_e[0][:], acc_e[1][:],
                                    ALU.add)
            nc.vector.tensor_tensor(es[:, 1:2], acc_e[2][:], acc_e[3][:],
                                    ALU.add)
            nc.vector.tensor_tensor(out_sb[:, 0:1], hs[:, 0:1], hs[:, 1:2],
                                    ALU.add)
            nc.vector.tensor_tensor(out_sb[:, 1:2], es[:, 0:1], es[:, 1:2],
                                    ALU.add)
            nc.sync.dma_start(out[:], out_sb[:])

    nc.compile()
    return nc


def _host_inputs(x: np.ndarray):
    """Shard + transpose + cast x into per-core input maps."""
    f = np.ascontiguousarray(x.reshape(N, K))
    groups = np.arange(N) // BK
    cols = np.arange(N)

    in_maps = []
    for c in range(N_CORES):
        ftc = np.ascontiguousarray(
            f[:, c * KC:(c + 1) * KC].T).astype(ml_dtypes.bfloat16)
        rows = c * 128 + np.arange(128)
        g_r = groups[rows]
        same = ((g_r[:, None] == groups[None, :]) &
                (rows[:, None] != cols[None, :])).astype(np.float32)
        diff = (g_r[:, None] != groups[None, :]).astype(np.float32)
        em = np.zeros((128, 8), dtype=np.float32)
        em[:, c] = 1.0
        in_maps.append({
            "ft": ftc.reshape(KT, 128, N),
            "mask_same": same,
            "mask_diff": diff,
            "emask": em,
        })
    return in_maps


def kernel(x: np.ndarray):
    if "nc" not in _CACHE:
        _CACHE["nc"] = _build_nc()
    nc = _CACHE["nc"]

    in_maps = _host_inputs(x)
    res = bass_utils.run_bass_kernel_spmd(
        nc, in_maps, core_ids=list(range(N_CORES)))

    total_h = 0.0
    total_e = 0.0
    for c in range(N_CORES):
        o = res.results[c]["out"].astype(np.float64)
        total_h += o[:, 0].sum()
        total_e += o[:, 1].sum()

    # reference: 2 * (0.5 * sum_same dist) / (N * (BK - 1))
    #            2 * sum_diff relu(1 - dist) / (N * (N // BK - 1))
    # device accumulates sum(min(dist-1, 0) * mask_diff) = -heter partial
    homo = total_h / (N * (BK - 1))
    heter = -2.0 * total_e / (N * (N // BK - 1))
    return (np.float32(homo), np.float32(heter))
